# revision 1
# baseline (speedup 1.0000x reference)
"""BitLinear158 forward on 8 Trainium2 NeuronCores.

y = x @ quantize(W).T where quantize is the absmean ternary quantizer:
    gamma = mean(|W|) + 1e-6 ; qw = sign(W) * min(round(|W/gamma|), 1)

Strategy (tensor parallel over out_features, x replicated):
  - host: compute the exact fp32 threshold T such that qw != 0 <=> |w| > T
    (T is derived from a bit-exact replication of the reference quantizer;
    the division-based quantizer is monotone in |w|, so a single scalar
    threshold reproduces it exactly on the given weight).
  - each core: DMA its W.T shard, quantize on-device to ternary bf16
    ({-1,0,1} are exact in bf16), then compute x @ qw.T with the PE array
    using a dual-bf16 split of x (x = hi + lo) accumulated in fp32 PSUM,
    which gives ~fp32 accuracy at 2 bf16 passes.
  - layouts are pre-transposed on host (xT, wT) so every DMA is a clean
    partition-major access pattern and no on-chip transposes are needed.
"""

import numpy as np

import concourse.bass as bass
import concourse.bacc as bacc
import concourse.mybir as mybir
import concourse.tile as tile
from concourse import bass_utils

# Problem shapes (hardcoded per contract).
B, S, D_IN, D_OUT = 4, 2048, 4096, 16384
N_CORES = 8
O_PER = D_OUT // N_CORES          # 2048 out-features per core
T_TOK = B * S                     # 8192 tokens
EPS = 1e-6

# Set by test harness to capture profiling info; leave False for grading.
TRACE = False
TMPDIR = None
LAST_RESULTS = None


def _quantize_ref(weight: np.ndarray) -> np.ndarray:
    """Bit-exact replication of reference.absmean_quantize (eager jax on the
    default backend, matching how the reference executes); numpy fallback."""
    try:
        import jax.numpy as jnp

        gamma = jnp.abs(weight).mean() + EPS
        ws = weight / gamma
        qw = jnp.sign(ws) * jnp.minimum(jnp.round(jnp.abs(ws)), 1.0)
        return np.asarray(qw)
    except Exception:
        gamma = np.float32(np.abs(weight).mean(dtype=np.float64)) + np.float32(EPS)
        ws = (weight / gamma).astype(np.float32)
        return (np.sign(ws) * np.minimum(np.round(np.abs(ws)), np.float32(1.0))
                ).astype(np.float32)


def _threshold(weight: np.ndarray) -> float:
    """Largest |w| that quantizes to 0. Then (|w| > T) <=> (qw != 0),
    exactly, by monotonicity of the quantizer in |w|."""
    qw = _quantize_ref(weight)
    aw = np.abs(weight)
    zeros = qw == 0
    if zeros.any():
        t = np.float32(aw[zeros].max())
    else:
        t = np.float32(0.0)
    mism = int(((aw > t) != (qw != 0)).sum())
    if mism:
        # should be impossible (monotonicity); threshold is still the best
        # separator, so proceed.
        print(f"kernel.py: threshold calibration mismatches: {mism}")
    return float(t)


def build_program(thresh: float, t_tiles: int = T_TOK // 128,
                  o_per: int = O_PER, ks: int = D_IN // 128) -> bass.Bass:
    """Emit the per-core Bass/Tile program.

    DRAM I/O (per core):
      xT [ks*128, t_tiles*128] f32  -- x transposed, replicated
      wT [ks*128, o_per]       f32  -- this core's W.T shard
      y  [t_tiles*128, o_per]  f32  -- this core's output slice
    """
    K = ks * 128
    T = t_tiles * 128
    NCHUNK = o_per // 512
    XH = 2                       # x staged in halves of ks//XH slabs
    HS = ks // XH

    nc = bacc.Bacc("TRN2", target_bir_lowering=False, debug=False)
    xT = nc.dram_tensor("xT", [K, T], mybir.dt.float32, kind="ExternalInput")
    wT = nc.dram_tensor("wT", [K, o_per], mybir.dt.float32,
                        kind="ExternalInput")
    y = nc.dram_tensor("y", [T, o_per], mybir.dt.float32,
                       kind="ExternalOutput")

    xTr = xT.ap().rearrange("(k p) t -> p k t", p=128)
    wTr = wT.ap().rearrange("(k p) o -> p k o", p=128)

    with tile.TileContext(nc) as tc:
        with (
            tc.tile_pool(name="qw", bufs=1) as qw_pool,
            tc.tile_pool(name="wstage", bufs=2) as wstage_pool,
            tc.tile_pool(name="xstage", bufs=1) as xstage_pool,
            tc.tile_pool(name="qtmp", bufs=1) as qtmp_pool,
            tc.tile_pool(name="xhi", bufs=2) as xhi_pool,
            tc.tile_pool(name="xlo", bufs=2) as xlo_pool,
            tc.tile_pool(name="outs", bufs=2) as out_pool,
            tc.tile_pool(name="psum", bufs=2, space="PSUM") as psum_pool,
        ):
            def x_convert(t):
                """Load+split one 128-token tile of xT into bf16 hi/lo."""
                xhi = xhi_pool.tile([128, ks, 128], mybir.dt.bfloat16,
                                    name="xhi", tag="xhi")
                xlo = xlo_pool.tile([128, ks, 128], mybir.dt.bfloat16,
                                    name="xlo", tag="xlo")
                for hf in range(XH):
                    xst = xstage_pool.tile([128, HS, 128], mybir.dt.float32,
                                           name="xst", tag="xst")
                    nc.gpsimd.dma_start(
                        out=xst,
                        in_=xTr[:, hf * HS:(hf + 1) * HS,
                                t * 128:(t + 1) * 128],
                    )
                    hs = slice(hf * HS, (hf + 1) * HS)
                    nc.vector.tensor_copy(out=xhi[:, hs, :], in_=xst)
                    # mixed-dtype subtract: f32 - bf16 -> bf16
                    nc.vector.tensor_tensor(
                        out=xlo[:, hs, :], in0=xst, in1=xhi[:, hs, :],
                        op=mybir.AluOpType.subtract,
                    )
                return xhi, xlo

            # First token tile's x conversion is emitted ahead of the
            # quantize loop so the PE can start as soon as slab 0 lands.
            xcur = x_convert(0)

            # ---- quantize weight shard to ternary bf16, kept resident ----
            qw = qw_pool.tile([128, ks, o_per], mybir.dt.bfloat16)
            for k in range(ks):
                wst = wstage_pool.tile([128, o_per], mybir.dt.float32)
                nc.gpsimd.dma_start(out=wst, in_=wTr[:, k, :])
                lt = qtmp_pool.tile([128, o_per], mybir.dt.bfloat16)
                nc.vector.tensor_scalar(
                    out=qw[:, k, :], in0=wst, scalar1=thresh, scalar2=None,
                    op0=mybir.AluOpType.is_gt,
                )
                nc.vector.tensor_scalar(
                    out=lt, in0=wst, scalar1=-thresh, scalar2=None,
                    op0=mybir.AluOpType.is_lt,
                )
                nc.vector.tensor_tensor(
                    out=qw[:, k, :], in0=qw[:, k, :], in1=lt,
                    op=mybir.AluOpType.subtract,
                )

            # ---- main loop over 128-token tiles ----
            for t in range(t_tiles):
                xhi, xlo = xcur
                if t + 1 < t_tiles:
                    xnext = x_convert(t + 1)

                ot = out_pool.tile([128, o_per], mybir.dt.float32)
                pss = [psum_pool.tile([128, 512], mybir.dt.float32,
                                      name=f"ps{c}", tag=f"ps{c}")
                       for c in range(NCHUNK)]
                for k in range(ks):
                    for h, xb in ((0, xhi), (1, xlo)):
                        for c in range(NCHUNK):
                            nc.tensor.matmul(
                                pss[c],
                                xb[:, k, :],
                                qw[:, k, c * 512:(c + 1) * 512],
                                start=(k == 0 and h == 0),
                                stop=(k == ks - 1 and h == 1),
                            )
                for c in range(NCHUNK):
                    nc.scalar.copy(out=ot[:, c * 512:(c + 1) * 512],
                                   in_=pss[c])
                nc.scalar.dma_start(
                    out=y.ap()[t * 128:(t + 1) * 128, :], in_=ot,
                )
                if t + 1 < t_tiles:
                    xcur = xnext
    nc.compile()
    return nc


def kernel(x: np.ndarray, weight: np.ndarray) -> np.ndarray:
    global LAST_RESULTS
    assert x.shape == (B, S, D_IN) and weight.shape == (D_OUT, D_IN)

    thresh = _threshold(weight)

    # Host-side layout prep: transpose for partition-major DMA.
    xT = np.ascontiguousarray(x.reshape(T_TOK, D_IN).T.astype(np.float32,
                                                              copy=False))
    wT = np.ascontiguousarray(weight.T.astype(np.float32, copy=False))

    nc = build_program(thresh)
    in_maps = [
        {"xT": xT,
         "wT": np.ascontiguousarray(wT[:, c * O_PER:(c + 1) * O_PER])}
        for c in range(N_CORES)
    ]
    res = bass_utils.run_bass_kernel_spmd(
        nc, in_maps, list(range(N_CORES)), trace=TRACE, tmpdir=TMPDIR,
    )
    LAST_RESULTS = res
    y = np.concatenate([res.results[c]["y"] for c in range(N_CORES)], axis=1)
    return np.ascontiguousarray(y.reshape(B, S, D_OUT).astype(np.float32,
                                                              copy=False))



# revision 5
# speedup vs baseline: 3.9744x; 3.9744x over previous
"""BitLinear158 forward on 8 Trainium2 NeuronCores.

y = x @ quantize(W).T where quantize is the absmean ternary quantizer:
    gamma = mean(|W|) + 1e-6 ; qw = sign(W) * min(round(|W/gamma|), 1)

Strategy (tensor parallel over out_features, x replicated):
  - host: replicate the reference quantizer bit-exactly (jax), producing
    the ternary weights; {-1,0,1} are exact in bf16/fp8, so the weight
    shard is shipped pre-quantized in the matmul dtype.
  - host: pre-layout x in the matmul dtype, tiled so every DMA line is
    a long contiguous per-partition read.
  - device: pure GEMM pipeline per core -- DMA x token-tiles + weight
    k-slabs, PE matmuls accumulating K in PSUM, scalar copies PSUM out,
    DMA the y slice back.

MODE selects the PE numerics (accuracy is checked against a 2e-2 gate):
  - "bf16": one bf16 pass over K=4096. rel err ~1.2e-3.
  - "fp8x2": x split hi+lo in fp8e4 (x = hi + lo exactly captures ~8
    mantissa bits), both stacked along K (K_eff=8192), consumed by
    DoubleRow matmuls (2 k-tiles per instruction). rel err ~2e-3.
  - "fp8": hi only (K_eff=4096), DoubleRow. rel err ~1.7e-2.
"""

import numpy as np
import ml_dtypes

import concourse.bass as bass
import concourse.bacc as bacc
import concourse.mybir as mybir
import concourse.tile as tile
from concourse import bass_utils

# Problem shapes (hardcoded per contract).
B, S, D_IN, D_OUT = 4, 2048, 4096, 16384
N_CORES = 8
O_PER = D_OUT // N_CORES          # 2048 out-features per core
T_TOK = B * S                     # 8192 tokens
T_TILES = T_TOK // 128            # 64 token tiles
KS = D_IN // 128                  # 32 k-slabs of 128
EPS = 1e-6

MODE = "bf16"                     # "bf16" | "fp8" | "fp8x2"

# Set by test harness to capture profiling info; leave False for grading.
TRACE = False
TMPDIR = None
LAST_RESULTS = None


def _quantize_ref(weight: np.ndarray) -> np.ndarray:
    """Bit-exact replication of reference.absmean_quantize (eager jax on the
    default backend, matching how the reference executes); numpy fallback."""
    try:
        import jax.numpy as jnp

        gamma = jnp.abs(weight).mean() + EPS
        ws = weight / gamma
        qw = jnp.sign(ws) * jnp.minimum(jnp.round(jnp.abs(ws)), 1.0)
        return np.asarray(qw)
    except Exception:
        gamma = np.float32(np.abs(weight).mean(dtype=np.float64)) + np.float32(EPS)
        ws = (weight / gamma).astype(np.float32)
        return (np.sign(ws) * np.minimum(np.round(np.abs(ws)), np.float32(1.0))
                ).astype(np.float32)


def build_program(mode: str) -> bass.Bass:
    """Emit the per-core Bass/Tile program.

    DRAM I/O (per core):
      xq [T_TILES, 128, ks_eff*128]  -- x tiles, [token-tile][K-part][k-slab*tok]
                                        so each partition line is one long
                                        contiguous read (ks_eff*128 elems)
      wq [ks*128, O_PER]             -- this core's ternary W.T shard
      y  [T_TOK, O_PER] f32          -- this core's output slice
    """
    fp8 = mode in ("fp8", "fp8x2")
    mdt = mybir.dt.float8e4 if fp8 else mybir.dt.bfloat16
    ks_eff = KS * 2 if mode == "fp8x2" else KS   # k-slabs fed to the PE

    nc = bacc.Bacc("TRN2", target_bir_lowering=False, debug=False)
    xq = nc.dram_tensor("xq", [T_TILES, 128, ks_eff * 128], mdt,
                        kind="ExternalInput")
    wq = nc.dram_tensor("wq", [KS * 128, O_PER], mdt, kind="ExternalInput")
    y = nc.dram_tensor("y", [T_TOK, O_PER], mybir.dt.float32,
                       kind="ExternalOutput")

    wr = wq.ap().rearrange("(k p) o -> p k o", p=128)

    with tile.TileContext(nc) as tc:
        with (
            tc.tile_pool(name="qw", bufs=1) as qw_pool,
            tc.tile_pool(name="xs", bufs=3) as x_pool,
            tc.tile_pool(name="outs", bufs=2) as out_pool,
            tc.tile_pool(name="psum", bufs=2, space="PSUM") as psum_pool,
        ):
            # Resident ternary weight shard, DMA'd k-slab by k-slab so the
            # first matmuls only wait on slab 0.
            qw = qw_pool.tile([128, KS, O_PER], mdt)
            for k in range(KS):
                nc.gpsimd.dma_start(out=qw[:, k, :], in_=wr[:, k, :])

            def fetch_x(t):
                xt = x_pool.tile([128, ks_eff, 128], mdt, name="xt", tag="xt")
                nc.sync.dma_start(out=xt, in_=xq.ap()[t].rearrange(
                    "p (k t) -> p k t", k=ks_eff))
                return xt

            xcur = fetch_x(0)
            for t in range(T_TILES):
                xt = xcur
                if t + 1 < T_TILES:
                    xcur = fetch_x(t + 1)

                pss = [psum_pool.tile([128, 512], mybir.dt.float32,
                                      name=f"ps{b}", tag=f"ps{b}")
                       for b in range(4)]
                if not fp8:
                    for k in range(KS):
                        for c in range(4):
                            nc.tensor.matmul(
                                pss[c], xt[:, k, :],
                                qw[:, k, c * 512:(c + 1) * 512],
                                start=(k == 0), stop=(k == KS - 1),
                            )
                else:
                    # DoubleRow: 2 k-slabs per matmul, 512-wide moving
                    # (probe-validated: natural [p,2,f] slices, exact).
                    nkk = ks_eff // 2
                    for kk in range(nkk):
                        k2 = (2 * kk) % KS      # wq reused for hi and lo
                        xsl = xt[:, 2 * kk:2 * kk + 2, :]
                        for c in range(4):
                            nc.tensor.matmul(
                                pss[c], xsl,
                                qw[:, k2:k2 + 2, c * 512:(c + 1) * 512],
                                start=(kk == 0), stop=(kk == nkk - 1),
                                perf_mode=mybir.MatmulPerfMode.DoubleRow,
                            )

                ot = out_pool.tile([128, O_PER], mybir.dt.float32)
                for b in range(4):
                    nc.scalar.copy(out=ot[:, b * 512:(b + 1) * 512],
                                   in_=pss[b])
                nc.scalar.dma_start(
                    out=y.ap()[t * 128:(t + 1) * 128, :], in_=ot,
                )
    nc.compile()
    return nc


def _tile_x(xk: np.ndarray) -> np.ndarray:
    """[K_eff, T] -> [T_TILES, 128, K_eff/128 * 128] so each SBUF partition
    line of a token-tile DMA is contiguous in DRAM."""
    ke = xk.shape[0]
    a = xk.reshape(ke // 128, 128, T_TILES, 128)      # [kb, p, tb, tw]
    return np.ascontiguousarray(
        a.transpose(2, 1, 0, 3).reshape(T_TILES, 128, ke))


def kernel(x: np.ndarray, weight: np.ndarray) -> np.ndarray:
    global LAST_RESULTS
    assert x.shape == (B, S, D_IN) and weight.shape == (D_OUT, D_IN)

    qw = _quantize_ref(weight).astype(np.float32)

    fp8 = MODE in ("fp8", "fp8x2")
    mdt = ml_dtypes.float8_e4m3 if fp8 else ml_dtypes.bfloat16

    xf = np.ascontiguousarray(x.reshape(T_TOK, D_IN).T)   # [K, T] f32
    if MODE == "fp8x2":
        hi = xf.astype(mdt)
        lo = (xf - hi.astype(np.float32)).astype(mdt)
        xm = np.concatenate([hi, lo], axis=0)             # [2K, T]
    else:
        xm = xf.astype(mdt)
    xq = _tile_x(xm)

    wT = np.ascontiguousarray(qw.T.astype(mdt))           # [K, D_OUT]

    nc = build_program(MODE)
    in_maps = [
        {"xq": xq,
         "wq": np.ascontiguousarray(wT[:, c * O_PER:(c + 1) * O_PER])}
        for c in range(N_CORES)
    ]
    res = bass_utils.run_bass_kernel_spmd(
        nc, in_maps, list(range(N_CORES)), trace=TRACE, tmpdir=TMPDIR,
    )
    LAST_RESULTS = res
    y = np.concatenate([res.results[c]["y"] for c in range(N_CORES)], axis=1)
    return np.ascontiguousarray(y.reshape(B, S, D_OUT).astype(np.float32,
                                                              copy=False))
